# revision 33
# baseline (speedup 1.0000x reference)
import os
import sys

sys.path.insert(0, "/opt/trn_rl_repo")

import numpy as np

import concourse.bacc as bacc
import concourse.bass as bass
import concourse.mybir as mybir
import concourse.tile as tile
from concourse.tile_rust import add_dep_helper
from concourse.masks import make_identity
from concourse.bass_utils import run_bass_kernel_spmd

N_CORES = 8
P = 128
OOB = 1 << 20  # sentinel index: skipped via bounds_check

# Set by test harness to capture a perfetto trace + exec time.
TRACE = False
DEBUG = False
LAST_EXEC_NS = None
LAST_RESULTS = None
LAST_PLAN = None


def _ceil_to(v, m):
    return (v + m - 1) // m * m


def _plan(x, Wg):
    """Host-side routing plan. Only integer index bookkeeping is derived here;
    every float that reaches the output is computed on device."""
    B, D = x.shape
    E = Wg.shape[1]
    EPC = E // N_CORES

    logits = x.astype(np.float64) @ Wg.astype(np.float64)
    order = np.argsort(-logits, axis=1, kind="stable")
    e1 = order[:, 0].astype(np.int64)
    e2 = order[:, 1].astype(np.int64)
    core_of = lambda e: e // EPC

    A_tok = [np.where(e1 == e)[0] for e in range(E)]  # this expert is top-1
    B_tok = [np.where(e2 == e)[0] for e in range(E)]  # this expert is top-2

    cnt = np.array([len(A_tok[e]) + len(B_tok[e]) for e in range(E)])
    CAP = int(max(_ceil_to(int(cnt.max()), P), P))

    # Slot order per expert block: [B slots (by dst core, then token)] [A slots]
    slot_tok = np.full((N_CORES, EPC * CAP), -1, np.int64)
    B_lists = [
        [[[] for _ in range(N_CORES)] for _ in range(EPC)] for _ in range(N_CORES)
    ]  # [src][block][dst] -> in-block slot rows
    A_rows = [[] for _ in range(N_CORES)]  # [core] -> (slot_row, token)
    for c in range(N_CORES):
        for b in range(EPC):
            e = EPC * c + b
            base = b * CAP
            i = 0
            bt = B_tok[e]
            dst = core_of(e1[bt])
            for d in range(N_CORES):
                for t in bt[dst == d]:
                    slot_tok[c, base + i] = t
                    B_lists[c][b][d].append(i)
                    i += 1
            # B zone must fit in the first slot group (all-to-all fires then)
            assert i <= min(1024, CAP), "B zone exceeds the first slot group"
            for t in A_tok[e]:
                slot_tok[c, base + i] = t
                A_rows[c].append((base + i, t))
                i += 1
            assert i <= CAP

    C4 = _ceil_to(
        max(
            len(B_lists[c][b][d])
            for c in range(N_CORES)
            for b in range(EPC)
            for d in range(N_CORES)
        ),
        16,
    )
    send_idx = np.zeros((N_CORES, EPC, N_CORES * C4), np.int32)
    recv_pos = {}  # (src, global slot row) -> recv row in merged recv tensor
    for c in range(N_CORES):
        for b in range(EPC):
            for d in range(N_CORES):
                for p, r in enumerate(B_lists[c][b][d]):
                    send_idx[c, b, d * C4 + p] = r
                    recv_pos[(c, b * CAP + r)] = b * N_CORES * C4 + c * C4 + p

    # Combine: out slot row -> recv row holding its partner contribution.
    S = EPC * CAP
    b_slot_of_tok = np.full(B, -1, np.int64)
    for c in range(N_CORES):
        for b in range(EPC):
            for d in range(N_CORES):
                for r in B_lists[c][b][d]:
                    b_slot_of_tok[slot_tok[c, b * CAP + r]] = b * CAP + r
    b_idx = np.full((N_CORES, S), OOB, np.int32)
    for c in range(N_CORES):
        for srow, t in A_rows[c]:
            src = core_of(e2[t])
            b_idx[c, srow] = recv_pos[(src, b_slot_of_tok[t])]

    # Send-side scatter: slot row (within block) -> send buffer position.
    T_B = max(
        (max((B_lists[c][b][d] or [0]) for d in range(N_CORES)) and 0) or 0
        for c in range(N_CORES) for b in range(EPC)
    )
    maxB = max(
        sum(len(B_lists[c][b][d]) for d in range(N_CORES))
        for c in range(N_CORES) for b in range(EPC)
    )
    T_B = (maxB + P - 1) // P
    s_scat = np.full((N_CORES, EPC, T_B * P), OOB, np.int32)
    for c in range(N_CORES):
        for b in range(EPC):
            for d in range(N_CORES):
                for p, r in enumerate(B_lists[c][b][d]):
                    s_scat[c, b, r] = d * C4 + p

    Acnt = np.array([len(A_rows[c]) for c in range(N_CORES)])
    return dict(
        E=E, EPC=EPC, CAP=CAP, C4=C4,
        e1=e1, e2=e2, slot_tok=slot_tok,
        send_idx=send_idx, b_idx=b_idx, s_scat=s_scat, T_B=T_B,
        A_rows=A_rows, Acnt=Acnt,
    )


def _build(nc, D, H, O, E, EPC, CAP, C4, T_B, add_b1, add_b2, debug=False):
    dt = mybir.dt
    S = EPC * CAP
    KD = D // P     # contraction chunks for layer 1 / gating
    KH = H // P     # contraction chunks for layer 2
    MH = H // P     # hid output chunks in layer 1
    NO2 = O // 512  # 512-wide output chunks in layer 2
    n_blk_tiles = CAP // P
    NQ = 4          # weight-load split

    xT = nc.dram_tensor("xT", [P, KD, S], dt.float32, kind="ExternalInput")
    Wg_in = nc.dram_tensor("Wg", [P, EPC, KD, E], dt.float32, kind="ExternalInput")
    W1_in = nc.dram_tensor("W1", [EPC, P, KD, H], dt.float32, kind="ExternalInput")
    W2_in = nc.dram_tensor("W2", [EPC, P, KH, O], dt.float32, kind="ExternalInput")
    b1_in = nc.dram_tensor("b1", [P, EPC, MH], dt.float32, kind="ExternalInput")
    b2_in = nc.dram_tensor("b2", [P, O], dt.float32, kind="ExternalInput")
    sidx_in = nc.dram_tensor("sidx", [P, EPC, T_B], dt.int32, kind="ExternalInput")
    bidx_in = nc.dram_tensor("bidx", [P, S // P], dt.int32, kind="ExternalInput")
    out = nc.dram_tensor("out", [S, O], dt.float32, kind="ExternalOutput")
    if debug:
        dbg_y = nc.dram_tensor("dbg_y", [S, O], dt.float32, kind="ExternalOutput")
        dbg_recv = nc.dram_tensor("dbg_recv", [EPC * N_CORES * C4, O], dt.float32, kind="ExternalOutput")
        dbg_cw = nc.dram_tensor("dbg_cw", [P, EPC * n_blk_tiles], dt.float32, kind="ExternalOutput")

    T_send = (N_CORES * C4) // P

    with tile.TileContext(nc) as tc:
        with (
            tc.tile_pool(name="dram", bufs=1, space="DRAM") as dram,
            tc.tile_pool(name="const", bufs=1) as constp,
            tc.tile_pool(name="wpool", bufs=1) as wpool,
            tc.tile_pool(name="xpool", bufs=2) as xpool,
            tc.tile_pool(name="hpool", bufs=1) as hpool,
            tc.tile_pool(name="ypool", bufs=3) as ypool,
            tc.tile_pool(name="gpool", bufs=2) as gpool,
            tc.tile_pool(name="cpool", bufs=2) as cpool,
            tc.tile_pool(name="psumg", bufs=1, space="PSUM") as psumg,
            tc.tile_pool(name="psumt", bufs=1, space="PSUM") as psumt,
            tc.tile_pool(name="psum1", bufs=4, space="PSUM") as psum1,
            tc.tile_pool(name="psum2", bufs=2, space="PSUM") as psum2,
        ):
            y_full = dram.tile([S, O], dt.float32, name="y_full")
            send_bufs = [dram.tile([N_CORES * C4, O], dt.bfloat16, name=f"send{b}") for b in range(EPC)]
            recv_all = dram.tile([EPC * N_CORES * C4, O], dt.bfloat16, name="recv_all")

            # ---- constants ----
            Wg_sb = constp.tile([P, EPC, KD, E], dt.bfloat16)
            nc.gpsimd.dma_start(Wg_sb[:], Wg_in[:])
            sidx_sb = constp.tile([P, EPC, T_B], dt.int32)
            nc.sync.dma_start(sidx_sb[:], sidx_in[:])
            bidx_sb = constp.tile([P, S // P], dt.int32)
            nc.sync.dma_start(bidx_sb[:], bidx_in[:])
            cw_sb = constp.tile([P, EPC * n_blk_tiles], dt.float32)
            ident = constp.tile([E, E], dt.float32)
            make_identity(nc, ident[:])
            if add_b1:
                b1_sb = constp.tile([P, EPC, MH], dt.float32)
                nc.sync.dma_start(b1_sb[:], b1_in[:])
            if add_b2:
                b2_sb = constp.tile([P, O], dt.float32)
                nc.sync.dma_start(b2_sb[:], b2_in[:])

            prev_wdma = None  # chain big loads so early chunks get full bandwidth
            for b in range(EPC):
                # ---- group-0 activations first, then weights (cast to bf16 in DMA) ----
                xb0 = xpool.tile([P, KD, 1024], dt.bfloat16, tag="xb", name=f"xb_{b}_0")
                gw0 = min(T_B * P, CAP)
                dma = nc.gpsimd.dma_start(xb0[:, :, :gw0], xT[:, :, b * CAP : b * CAP + gw0])
                if prev_wdma is not None:
                    add_dep_helper(dma.ins, prev_wdma.ins, sync=True, reason="dma chain")
                prev_wdma = dma
                W1_qs = []
                for q in range(NQ):
                    w1q = wpool.tile([P, KD, H // NQ], dt.bfloat16, tag=f"w1q{q}", name=f"w1_{b}_{q}")
                    dma = nc.gpsimd.dma_start(
                        w1q[:], W1_in[b, :, :, q * (H // NQ) : (q + 1) * (H // NQ)]
                    )
                    add_dep_helper(dma.ins, prev_wdma.ins, sync=True, reason="dma chain")
                    prev_wdma = dma
                    W1_qs.append(w1q)
                W2_sb = wpool.tile([P, KH, O], dt.bfloat16, tag="w2", name=f"w2_{b}")
                for q in range(NQ):
                    dma = nc.gpsimd.dma_start(
                        W2_sb[:, q * (KH // NQ) : (q + 1) * (KH // NQ), :],
                        W2_in[b, :, q * (KH // NQ) : (q + 1) * (KH // NQ), :],
                    )
                    add_dep_helper(dma.ins, prev_wdma.ins, sync=True, reason="dma chain")
                    prev_wdma = dma

                GB = T_B * P  # first group covers exactly the B zone
                bounds = [0, GB] + list(range(GB + 512, CAP, 512)) + [CAP]
                bounds = sorted(set(b_ for b_ in bounds if b_ <= CAP))
                for gi in range(len(bounds) - 1):
                    g0 = bounds[gi]
                    gw = bounds[gi + 1] - g0
                    nsc = (gw + 511) // 512
                    if g0 == 0:
                        xb = xb0
                    else:
                        xb = xpool.tile([P, KD, 1024], dt.bfloat16, tag="xb", name=f"xb_{b}_{g0}")
                        xd = nc.gpsimd.dma_start(
                            xb[:, :, :gw], xT[:, :, b * CAP + g0 : b * CAP + g0 + gw]
                        )
                        add_dep_helper(xd.ins, prev_wdma.ins, sync=True, reason="dma chain")
                    # ---- gating (transposed): logitsT[E, w] chunks -> cw ----
                    for j in range(nsc):
                        w = min(512, gw - j * 512)
                        pgT = psumg.tile([E, 512], dt.float32, space="PSUM", tag="pgT", name=f"pgT_{b}_{g0}_{j}")
                        for k in range(KD):
                            nc.tensor.matmul(
                                pgT[:, :w], lhsT=Wg_sb[:, b, k, :], rhs=xb[:, k, j * 512 : j * 512 + w],
                                start=(k == 0), stop=(k == KD - 1),
                            )
                        lgT = gpool.tile([E, 512], dt.float32, tag="lgT", name=f"lgT_{b}_{g0}_{j}")
                        nc.vector.tensor_copy(lgT[:, :w], pgT[:, :w])
                        # per slot-tile: PE-transpose logits to token-major, then
                        # cw = sigmoid(l_own - max(others)) on DVE/ACT
                        for tt in range(w // P):
                            tps = psumt.tile([P, E], dt.float32, space="PSUM", tag="ptr", name=f"ptr_{b}_{g0}_{j}_{tt}")
                            nc.tensor.transpose(
                                tps[:], lgT[:, tt * P : (tt + 1) * P], ident[:]
                            )
                            Lt = gpool.tile([P, E], dt.float32, tag="Lt", name=f"Lt_{b}_{g0}_{j}_{tt}")
                            nc.vector.tensor_copy(Lt[:], tps[:])
                            Ltm = gpool.tile([P, E], dt.float32, tag="Ltm", name=f"Ltm_{b}_{g0}_{j}_{tt}")
                            nc.vector.tensor_copy(Ltm[:], tps[:])
                            nc.vector.memset(Ltm[:, 0:1], -1e30)
                            bmax = gpool.tile([P, 1], dt.float32, tag="bmax", name=f"bm_{b}_{g0}_{j}_{tt}")
                            nc.vector.tensor_reduce(
                                bmax[:], Ltm[:], axis=mybir.AxisListType.X, op=mybir.AluOpType.max
                            )
                            dlog = gpool.tile([P, 1], dt.float32, tag="dlog", name=f"dl_{b}_{g0}_{j}_{tt}")
                            nc.vector.tensor_sub(dlog[:], Lt[:, 0:1], bmax[:])
                            col = b * n_blk_tiles + (g0 + j * 512) // P + tt
                            nc.scalar.activation(
                                cw_sb[:, col : col + 1], dlog[:],
                                mybir.ActivationFunctionType.Sigmoid,
                            )

                    # ---- layer 1: h = relu(W1.T x) (feature-major) ----
                    h_sb = hpool.tile([P, MH, 1024], dt.bfloat16, tag="h", name=f"h_{b}_{g0}")
                    for m in range(MH):
                        ps = [
                            psum1.tile([P, 512], dt.float32, space="PSUM", tag="p1", name=f"p1_{b}_{g0}_{m}_{j}")
                            for j in range(nsc)
                        ]
                        mq = m // (MH // NQ)
                        mr = m % (MH // NQ)
                        for k in range(KD):
                            for j in range(nsc):
                                w = min(512, gw - j * 512)
                                nc.tensor.matmul(
                                    ps[j][:, :w],
                                    lhsT=W1_qs[mq][:, k, mr * P : (mr + 1) * P],
                                    rhs=xb[:, k, j * 512 : j * 512 + w],
                                    start=(k == 0), stop=(k == KD - 1),
                                )
                        for j in range(nsc):
                            w = min(512, gw - j * 512)
                            if add_b1:
                                nc.scalar.activation(
                                    h_sb[:, m, j * 512 : j * 512 + w], ps[j][:, :w],
                                    mybir.ActivationFunctionType.Relu,
                                    bias=b1_sb[:, b, m : m + 1],
                                )
                            else:
                                nc.scalar.activation(
                                    h_sb[:, m, j * 512 : j * 512 + w], ps[j][:, :w],
                                    mybir.ActivationFunctionType.Relu,
                                )
                    # ---- layer 2: y = cw * (W2.T h) (token-major) ----
                    for st in range(gw // P):
                        ssl = slice(st * P, (st + 1) * P)
                        col = b * n_blk_tiles + g0 // P + st
                        yt = ypool.tile([P, O], dt.float32, tag="y", name=f"y_{b}_{g0}_{st}")
                        pys = [
                            psum2.tile([P, 512], dt.float32, space="PSUM", tag="p2", name=f"p2_{b}_{g0}_{st}_{o}")
                            for o in range(NO2)
                        ]
                        for m in range(KH):
                            for o in range(NO2):
                                nc.tensor.matmul(
                                    pys[o][:],
                                    lhsT=h_sb[:, m, ssl],
                                    rhs=W2_sb[:, m, o * 512 : (o + 1) * 512],
                                    start=(m == 0), stop=(m == KH - 1),
                                )
                        for o in range(NO2):
                            nc.scalar.activation(
                                yt[:, o * 512 : (o + 1) * 512], pys[o][:],
                                mybir.ActivationFunctionType.Copy,
                                scale=cw_sb[:, col : col + 1],
                            )
                        if add_b2:
                            nc.vector.tensor_add(yt[:], yt[:], b2_sb[:])
                        row0 = b * CAP + g0 + st * P
                        nc.sync.dma_start(y_full[row0 : row0 + P, :], yt[:])
                        gst = g0 // P + st
                        if gst < T_B:
                            # scatter B rows straight into the send buffer (bf16)
                            ybt = ypool.tile([P, O], dt.bfloat16, tag="ybt", name=f"ybt_{b}_{g0}_{st}")
                            nc.vector.tensor_copy(ybt[:], yt[:])
                            nc.gpsimd.indirect_dma_start(
                                out=send_bufs[b][:],
                                out_offset=bass.IndirectOffsetOnAxis(ap=sidx_sb[:, b, gst : gst + 1], axis=0),
                                in_=ybt[:],
                                in_offset=None,
                                bounds_check=N_CORES * C4 - 1,
                                oob_is_err=False,
                            )
                        if gst == T_B - 1:
                            nc.gpsimd.collective_compute(
                                "AllToAll",
                                mybir.AluOpType.bypass,
                                replica_groups=[list(range(N_CORES))],
                                ins=[send_bufs[b].opt()],
                                outs=[recv_all[b * N_CORES * C4 : (b + 1) * N_CORES * C4, :]],
                            )

            # ---- combine: out[slot] = y_full[slot] + recv[b_idx[slot]] ----
            for t in range(S // P):
                at = cpool.tile([P, O], dt.float32, tag="at", name=f"at_{t}", bufs=6)
                nc.sync.dma_start(at[:], y_full[t * P : (t + 1) * P, :])
                bt = cpool.tile([P, O], dt.bfloat16, tag="bt", name=f"bt_{t}", bufs=6)
                nc.gpsimd.indirect_dma_start(
                    out=bt[:], out_offset=None,
                    in_=recv_all[:],
                    in_offset=bass.IndirectOffsetOnAxis(ap=bidx_sb[:, t : t + 1], axis=0),
                    bounds_check=EPC * N_CORES * C4 - 1,
                    oob_is_err=False,
                )
                nc.vector.tensor_add(at[:], at[:], bt[:])
                nc.scalar.dma_start(out[t * P : (t + 1) * P, :], at[:])

            if debug:
                nc.sync.dma_start(dbg_recv[:], recv_all[:])
                nc.sync.dma_start(dbg_cw[:], cw_sb[:])

    return out


def kernel(x, Wg, W1, b1, W2, b2):
    global LAST_EXEC_NS, LAST_RESULTS, LAST_PLAN
    x = np.ascontiguousarray(np.asarray(x, np.float32))
    Wg = np.ascontiguousarray(np.asarray(Wg, np.float32))
    W1 = np.ascontiguousarray(np.asarray(W1, np.float32))
    b1 = np.ascontiguousarray(np.asarray(b1, np.float32))
    W2 = np.ascontiguousarray(np.asarray(W2, np.float32))
    b2 = np.ascontiguousarray(np.asarray(b2, np.float32))

    B, D = x.shape
    E, _, H = W1.shape
    O = W2.shape[2]
    EPC = E // N_CORES

    pl = _plan(x, Wg)
    CAP, C4 = pl["CAP"], pl["C4"]
    S = EPC * CAP

    add_b1 = bool(np.any(b1))
    add_b2 = bool(np.any(b2))
    if add_b2:
        assert np.all(b2 == b2[0]), "per-expert nonzero b2 not supported"

    nc = bacc.Bacc("TRN2", target_bir_lowering=False, debug=False, num_devices=N_CORES)
    _build(nc, D, H, O, E, EPC, CAP, C4, pl["T_B"], add_b1, add_b2, debug=DEBUG)
    nc.compile()

    # ---- per-core input staging (pure data movement) ----
    xT_full = np.ascontiguousarray(x.T)  # [D, B]
    in_maps = []
    for c in range(N_CORES):
        toks = pl["slot_tok"][c]
        xTp = np.zeros((D, S), np.float32)
        real = toks >= 0
        xTp[:, real] = xT_full[:, toks[real]]
        xTp = np.ascontiguousarray(xTp.reshape(D // P, P, S).transpose(1, 0, 2))

        Wg_blocks = []
        for b in range(EPC):
            e = EPC * c + b
            perm = np.concatenate([[e], [j for j in range(E) if j != e]])
            Wg_blocks.append(Wg[:, perm].reshape(D // P, P, E).transpose(1, 0, 2))
        Wg_c = np.ascontiguousarray(np.stack(Wg_blocks, axis=1), np.float32)

        W1_c = np.stack(
            [W1[EPC * c + b].reshape(D // P, P, H).transpose(1, 0, 2) for b in range(EPC)]
        )
        W2_c = np.stack(
            [W2[EPC * c + b].reshape(H // P, P, O).transpose(1, 0, 2) for b in range(EPC)]
        )
        b1_c = np.stack([b1[EPC * c + b].reshape(H // P, P).T for b in range(EPC)]).transpose(1, 0, 2)
        b2_c = np.broadcast_to(b2[0], (P, O)).copy() if add_b2 else np.zeros((P, O), np.float32)
        in_maps.append(
            {
                "xT": np.ascontiguousarray(xTp, np.float32),
                "Wg": Wg_c,
                "W1": np.ascontiguousarray(W1_c, np.float32),
                "W2": np.ascontiguousarray(W2_c, np.float32),
                "b1": np.ascontiguousarray(b1_c, np.float32),
                "b2": np.ascontiguousarray(b2_c, np.float32),
                "sidx": np.ascontiguousarray(
                    pl["s_scat"][c].reshape(EPC, -1, P).transpose(2, 0, 1), np.int32
                ),
                "bidx": np.ascontiguousarray(
                    pl["b_idx"][c].reshape(-1, P).T, np.int32
                ),
            }
        )

    kwargs = {}
    if TRACE:
        import types

        try:
            import antenv  # noqa: F401
            from trn_agent_boot.trn_boot import _ntff_profile_via_ctypes

            hook = _ntff_profile_via_ctypes("/opt/axon/libaxon_pjrt.so")
            mod = types.ModuleType("antenv.axon_hooks")
            mod.get_axon_ntff_profile_hook = lambda: hook
            mod.set_axon_ntff_profile_hook = lambda h: None
            sys.modules.setdefault("antenv.axon_hooks", mod)
            kwargs["trace"] = True
        except Exception as e:  # pragma: no cover
            print("trace hook unavailable:", e)

    res = run_bass_kernel_spmd(nc, in_maps, core_ids=list(range(N_CORES)), **kwargs)
    LAST_EXEC_NS = res.exec_time_ns
    LAST_RESULTS = res.results
    LAST_PLAN = pl

    final = np.zeros((B, O), np.float32)
    for c in range(N_CORES):
        o = res.results[c]["out"]
        rows = np.array([sr for sr, _ in pl["A_rows"][c]], np.int64)
        tokens = np.array([t for _, t in pl["A_rows"][c]], np.int64)
        final[tokens] = o[rows]
    return final


# revision 34
# speedup vs baseline: 1.0595x; 1.0595x over previous
import os
import sys

sys.path.insert(0, "/opt/trn_rl_repo")

import numpy as np

import concourse.bacc as bacc
import concourse.bass as bass
import concourse.mybir as mybir
import concourse.tile as tile
from concourse.tile_rust import add_dep_helper
from concourse.masks import make_identity
from concourse.bass_utils import run_bass_kernel_spmd

N_CORES = 8
P = 128
OOB = 1 << 20  # sentinel index: skipped via bounds_check

# Set by test harness to capture a perfetto trace + exec time.
TRACE = False
DEBUG = False
LAST_EXEC_NS = None
LAST_RESULTS = None
LAST_PLAN = None


def _ceil_to(v, m):
    return (v + m - 1) // m * m


def _plan(x, Wg):
    """Host-side routing plan. Only integer index bookkeeping is derived here;
    every float that reaches the output is computed on device."""
    B, D = x.shape
    E = Wg.shape[1]
    EPC = E // N_CORES

    logits = x.astype(np.float64) @ Wg.astype(np.float64)
    order = np.argsort(-logits, axis=1, kind="stable")
    e1 = order[:, 0].astype(np.int64)
    e2 = order[:, 1].astype(np.int64)
    core_of = lambda e: e // EPC

    A_tok = [np.where(e1 == e)[0] for e in range(E)]  # this expert is top-1
    B_tok = [np.where(e2 == e)[0] for e in range(E)]  # this expert is top-2

    cnt = np.array([len(A_tok[e]) + len(B_tok[e]) for e in range(E)])
    CAP = int(max(_ceil_to(int(cnt.max()), P), P))

    # Slot order per expert block: [B slots (by dst core, then token)] [A slots]
    slot_tok = np.full((N_CORES, EPC * CAP), -1, np.int64)
    B_lists = [
        [[[] for _ in range(N_CORES)] for _ in range(EPC)] for _ in range(N_CORES)
    ]  # [src][block][dst] -> in-block slot rows
    A_rows = [[] for _ in range(N_CORES)]  # [core] -> (slot_row, token)
    for c in range(N_CORES):
        for b in range(EPC):
            e = EPC * c + b
            base = b * CAP
            i = 0
            bt = B_tok[e]
            dst = core_of(e1[bt])
            for d in range(N_CORES):
                for t in bt[dst == d]:
                    slot_tok[c, base + i] = t
                    B_lists[c][b][d].append(i)
                    i += 1
            # B zone must fit in the first slot group (all-to-all fires then)
            assert i <= min(1024, CAP), "B zone exceeds the first slot group"
            for t in A_tok[e]:
                slot_tok[c, base + i] = t
                A_rows[c].append((base + i, t))
                i += 1
            assert i <= CAP

    C4 = _ceil_to(
        max(
            len(B_lists[c][b][d])
            for c in range(N_CORES)
            for b in range(EPC)
            for d in range(N_CORES)
        ),
        16,
    )
    send_idx = np.zeros((N_CORES, EPC, N_CORES * C4), np.int32)
    recv_pos = {}  # (src, global slot row) -> recv row in merged recv tensor
    for c in range(N_CORES):
        for b in range(EPC):
            for d in range(N_CORES):
                for p, r in enumerate(B_lists[c][b][d]):
                    send_idx[c, b, d * C4 + p] = r
                    recv_pos[(c, b * CAP + r)] = b * N_CORES * C4 + c * C4 + p

    # Combine: out slot row -> recv row holding its partner contribution.
    S = EPC * CAP
    b_slot_of_tok = np.full(B, -1, np.int64)
    for c in range(N_CORES):
        for b in range(EPC):
            for d in range(N_CORES):
                for r in B_lists[c][b][d]:
                    b_slot_of_tok[slot_tok[c, b * CAP + r]] = b * CAP + r
    b_idx = np.full((N_CORES, S), OOB, np.int32)
    for c in range(N_CORES):
        for srow, t in A_rows[c]:
            src = core_of(e2[t])
            b_idx[c, srow] = recv_pos[(src, b_slot_of_tok[t])]

    # Send-side scatter: slot row (within block) -> send buffer position.
    T_B = max(
        (max((B_lists[c][b][d] or [0]) for d in range(N_CORES)) and 0) or 0
        for c in range(N_CORES) for b in range(EPC)
    )
    maxB = max(
        sum(len(B_lists[c][b][d]) for d in range(N_CORES))
        for c in range(N_CORES) for b in range(EPC)
    )
    T_B = (maxB + P - 1) // P
    s_scat = np.full((N_CORES, EPC, T_B * P), OOB, np.int32)
    for c in range(N_CORES):
        for b in range(EPC):
            for d in range(N_CORES):
                for p, r in enumerate(B_lists[c][b][d]):
                    s_scat[c, b, r] = d * C4 + p

    Acnt = np.array([len(A_rows[c]) for c in range(N_CORES)])
    return dict(
        E=E, EPC=EPC, CAP=CAP, C4=C4,
        e1=e1, e2=e2, slot_tok=slot_tok,
        send_idx=send_idx, b_idx=b_idx, s_scat=s_scat, T_B=T_B,
        A_rows=A_rows, Acnt=Acnt,
    )


def _build(nc, D, H, O, E, EPC, CAP, C4, T_B, add_b1, add_b2, debug=False):
    dt = mybir.dt
    S = EPC * CAP
    KD = D // P     # contraction chunks for layer 1 / gating
    KH = H // P     # contraction chunks for layer 2
    MH = H // P     # hid output chunks in layer 1
    NO2 = O // 512  # 512-wide output chunks in layer 2
    n_blk_tiles = CAP // P
    NQ = 4          # weight-load split

    xT = nc.dram_tensor("xT", [P, KD, S], dt.float32, kind="ExternalInput")
    Wg_in = nc.dram_tensor("Wg", [P, EPC, KD, E], dt.float32, kind="ExternalInput")
    W1_in = nc.dram_tensor("W1", [EPC, P, KD, H], dt.float32, kind="ExternalInput")
    W2_in = nc.dram_tensor("W2", [EPC, P, KH, O], dt.float32, kind="ExternalInput")
    b1_in = nc.dram_tensor("b1", [P, EPC, MH], dt.float32, kind="ExternalInput")
    b2_in = nc.dram_tensor("b2", [P, O], dt.float32, kind="ExternalInput")
    sidx_in = nc.dram_tensor("sidx", [P, EPC, T_B], dt.int32, kind="ExternalInput")
    bidx_in = nc.dram_tensor("bidx", [P, S // P], dt.int32, kind="ExternalInput")
    out = nc.dram_tensor("out", [S, O], dt.float32, kind="ExternalOutput")
    if debug:
        dbg_y = nc.dram_tensor("dbg_y", [S, O], dt.float32, kind="ExternalOutput")
        dbg_recv = nc.dram_tensor("dbg_recv", [EPC * N_CORES * C4, O], dt.float32, kind="ExternalOutput")
        dbg_cw = nc.dram_tensor("dbg_cw", [P, EPC * n_blk_tiles], dt.float32, kind="ExternalOutput")

    T_send = (N_CORES * C4) // P

    with tile.TileContext(nc) as tc:
        with (
            tc.tile_pool(name="dram", bufs=1, space="DRAM") as dram,
            tc.tile_pool(name="const", bufs=1) as constp,
            tc.tile_pool(name="wpool", bufs=1) as wpool,
            tc.tile_pool(name="xpool", bufs=2) as xpool,
            tc.tile_pool(name="hpool", bufs=1) as hpool,
            tc.tile_pool(name="ypool", bufs=3) as ypool,
            tc.tile_pool(name="gpool", bufs=2) as gpool,
            tc.tile_pool(name="cpool", bufs=2) as cpool,
            tc.tile_pool(name="psumg", bufs=1, space="PSUM") as psumg,
            tc.tile_pool(name="psumt", bufs=1, space="PSUM") as psumt,
            tc.tile_pool(name="psum1", bufs=3, space="PSUM") as psum1,
            tc.tile_pool(name="psum2", bufs=3, space="PSUM") as psum2,
        ):
            y_full = dram.tile([S, O], dt.float32, name="y_full")
            send_bufs = [dram.tile([N_CORES * C4, O], dt.bfloat16, name=f"send{b}") for b in range(EPC)]
            recv_all = dram.tile([EPC * N_CORES * C4, O], dt.bfloat16, name="recv_all")

            # ---- constants ----
            Wg_sb = constp.tile([P, EPC, KD, E], dt.bfloat16)
            nc.gpsimd.dma_start(Wg_sb[:], Wg_in[:])
            sidx_sb = constp.tile([P, EPC, T_B], dt.int32)
            nc.sync.dma_start(sidx_sb[:], sidx_in[:])
            bidx_sb = constp.tile([P, S // P], dt.int32)
            nc.sync.dma_start(bidx_sb[:], bidx_in[:])
            cw_sb = constp.tile([P, EPC * n_blk_tiles], dt.float32)
            ident = constp.tile([E, E], dt.float32)
            make_identity(nc, ident[:])
            if add_b1:
                b1_sb = constp.tile([P, EPC, MH], dt.float32)
                nc.sync.dma_start(b1_sb[:], b1_in[:])
            if add_b2:
                b2_sb = constp.tile([P, O], dt.float32)
                nc.sync.dma_start(b2_sb[:], b2_in[:])

            prev_wdma = None  # chain big loads so early chunks get full bandwidth
            for b in range(EPC):
                # ---- group-0 activations first, then weights (cast to bf16 in DMA) ----
                xb0 = xpool.tile([P, KD, 1024], dt.bfloat16, tag="xb", name=f"xb_{b}_0")
                gw0 = min(T_B * P, CAP)
                dma = nc.gpsimd.dma_start(xb0[:, :, :gw0], xT[:, :, b * CAP : b * CAP + gw0])
                if prev_wdma is not None:
                    add_dep_helper(dma.ins, prev_wdma.ins, sync=True, reason="dma chain")
                prev_wdma = dma
                W1_qs = []
                for q in range(NQ):
                    w1q = wpool.tile([P, KD, H // NQ], dt.bfloat16, tag=f"w1q{q}", name=f"w1_{b}_{q}")
                    dma = nc.gpsimd.dma_start(
                        w1q[:], W1_in[b, :, :, q * (H // NQ) : (q + 1) * (H // NQ)]
                    )
                    add_dep_helper(dma.ins, prev_wdma.ins, sync=True, reason="dma chain")
                    prev_wdma = dma
                    W1_qs.append(w1q)
                W2_sb = wpool.tile([P, KH, O], dt.bfloat16, tag="w2", name=f"w2_{b}")
                for q in range(NQ):
                    dma = nc.gpsimd.dma_start(
                        W2_sb[:, q * (KH // NQ) : (q + 1) * (KH // NQ), :],
                        W2_in[b, :, q * (KH // NQ) : (q + 1) * (KH // NQ), :],
                    )
                    add_dep_helper(dma.ins, prev_wdma.ins, sync=True, reason="dma chain")
                    prev_wdma = dma

                GB = T_B * P  # first group covers exactly the B zone
                bounds = [0, GB] + list(range(GB + 512, CAP, 512)) + [CAP]
                bounds = sorted(set(b_ for b_ in bounds if b_ <= CAP))
                for gi in range(len(bounds) - 1):
                    g0 = bounds[gi]
                    gw = bounds[gi + 1] - g0
                    nsc = (gw + 511) // 512
                    if g0 == 0:
                        xb = xb0
                    else:
                        xb = xpool.tile([P, KD, 1024], dt.bfloat16, tag="xb", name=f"xb_{b}_{g0}")
                        xd = nc.gpsimd.dma_start(
                            xb[:, :, :gw], xT[:, :, b * CAP + g0 : b * CAP + g0 + gw]
                        )
                        add_dep_helper(xd.ins, prev_wdma.ins, sync=True, reason="dma chain")
                    # ---- gating (transposed): logitsT[E, w] chunks -> cw ----
                    for j in range(nsc):
                        w = min(512, gw - j * 512)
                        pgT = psumg.tile([E, 512], dt.float32, space="PSUM", tag="pgT", name=f"pgT_{b}_{g0}_{j}")
                        for k in range(KD):
                            nc.tensor.matmul(
                                pgT[:, :w], lhsT=Wg_sb[:, b, k, :], rhs=xb[:, k, j * 512 : j * 512 + w],
                                start=(k == 0), stop=(k == KD - 1),
                            )
                        lgT = gpool.tile([E, 512], dt.float32, tag="lgT", name=f"lgT_{b}_{g0}_{j}")
                        nc.vector.tensor_copy(lgT[:, :w], pgT[:, :w])
                        # per slot-tile: PE-transpose logits to token-major, then
                        # cw = sigmoid(l_own - max(others)) on DVE/ACT
                        for tt in range(w // P):
                            tps = psumt.tile([P, E], dt.float32, space="PSUM", tag="ptr", name=f"ptr_{b}_{g0}_{j}_{tt}")
                            nc.tensor.transpose(
                                tps[:], lgT[:, tt * P : (tt + 1) * P], ident[:]
                            )
                            Lt = gpool.tile([P, E], dt.float32, tag="Lt", name=f"Lt_{b}_{g0}_{j}_{tt}")
                            nc.vector.tensor_copy(Lt[:], tps[:])
                            Ltm = gpool.tile([P, E], dt.float32, tag="Ltm", name=f"Ltm_{b}_{g0}_{j}_{tt}")
                            nc.vector.tensor_copy(Ltm[:], tps[:])
                            nc.vector.memset(Ltm[:, 0:1], -1e30)
                            bmax = gpool.tile([P, 1], dt.float32, tag="bmax", name=f"bm_{b}_{g0}_{j}_{tt}")
                            nc.vector.tensor_reduce(
                                bmax[:], Ltm[:], axis=mybir.AxisListType.X, op=mybir.AluOpType.max
                            )
                            dlog = gpool.tile([P, 1], dt.float32, tag="dlog", name=f"dl_{b}_{g0}_{j}_{tt}")
                            nc.vector.tensor_sub(dlog[:], Lt[:, 0:1], bmax[:])
                            col = b * n_blk_tiles + (g0 + j * 512) // P + tt
                            nc.scalar.activation(
                                cw_sb[:, col : col + 1], dlog[:],
                                mybir.ActivationFunctionType.Sigmoid,
                            )

                    # ---- layer 1: h = relu(W1.T x) (feature-major) ----
                    h_sb = hpool.tile([P, MH, 1024], dt.bfloat16, tag="h", name=f"h_{b}_{g0}")
                    for m in range(MH):
                        ps = [
                            psum1.tile([P, 512], dt.float32, space="PSUM", tag="p1", name=f"p1_{b}_{g0}_{m}_{j}")
                            for j in range(nsc)
                        ]
                        mq = m // (MH // NQ)
                        mr = m % (MH // NQ)
                        for k in range(KD):
                            for j in range(nsc):
                                w = min(512, gw - j * 512)
                                nc.tensor.matmul(
                                    ps[j][:, :w],
                                    lhsT=W1_qs[mq][:, k, mr * P : (mr + 1) * P],
                                    rhs=xb[:, k, j * 512 : j * 512 + w],
                                    start=(k == 0), stop=(k == KD - 1),
                                )
                        for j in range(nsc):
                            w = min(512, gw - j * 512)
                            if add_b1:
                                nc.scalar.activation(
                                    h_sb[:, m, j * 512 : j * 512 + w], ps[j][:, :w],
                                    mybir.ActivationFunctionType.Relu,
                                    bias=b1_sb[:, b, m : m + 1],
                                )
                            else:
                                nc.scalar.activation(
                                    h_sb[:, m, j * 512 : j * 512 + w], ps[j][:, :w],
                                    mybir.ActivationFunctionType.Relu,
                                )
                    # ---- layer 2: y = cw * (W2.T h) (token-major) ----
                    for st in range(gw // P):
                        ssl = slice(st * P, (st + 1) * P)
                        col = b * n_blk_tiles + g0 // P + st
                        yt = ypool.tile([P, O], dt.float32, tag="y", name=f"y_{b}_{g0}_{st}")
                        pys = [
                            psum2.tile([P, 512], dt.float32, space="PSUM", tag="p2", name=f"p2_{b}_{g0}_{st}_{o}")
                            for o in range(NO2)
                        ]
                        for m in range(KH):
                            for o in range(NO2):
                                nc.tensor.matmul(
                                    pys[o][:],
                                    lhsT=h_sb[:, m, ssl],
                                    rhs=W2_sb[:, m, o * 512 : (o + 1) * 512],
                                    start=(m == 0), stop=(m == KH - 1),
                                )
                        for o in range(NO2):
                            nc.scalar.activation(
                                yt[:, o * 512 : (o + 1) * 512], pys[o][:],
                                mybir.ActivationFunctionType.Copy,
                                scale=cw_sb[:, col : col + 1],
                            )
                        if add_b2:
                            nc.vector.tensor_add(yt[:], yt[:], b2_sb[:])
                        row0 = b * CAP + g0 + st * P
                        nc.sync.dma_start(y_full[row0 : row0 + P, :], yt[:])
                        gst = g0 // P + st
                        if gst < T_B:
                            # scatter B rows straight into the send buffer (bf16)
                            ybt = ypool.tile([P, O], dt.bfloat16, tag="ybt", name=f"ybt_{b}_{g0}_{st}")
                            nc.vector.tensor_copy(ybt[:], yt[:])
                            nc.gpsimd.indirect_dma_start(
                                out=send_bufs[b][:],
                                out_offset=bass.IndirectOffsetOnAxis(ap=sidx_sb[:, b, gst : gst + 1], axis=0),
                                in_=ybt[:],
                                in_offset=None,
                                bounds_check=N_CORES * C4 - 1,
                                oob_is_err=False,
                            )
                        if gst == T_B - 1:
                            nc.gpsimd.collective_compute(
                                "AllToAll",
                                mybir.AluOpType.bypass,
                                replica_groups=[list(range(N_CORES))],
                                ins=[send_bufs[b].opt()],
                                outs=[recv_all[b * N_CORES * C4 : (b + 1) * N_CORES * C4, :]],
                            )

            # ---- combine: out[slot] = y_full[slot] + recv[b_idx[slot]] ----
            for t in range(S // P):
                at = cpool.tile([P, O], dt.float32, tag="at", name=f"at_{t}", bufs=6)
                nc.sync.dma_start(at[:], y_full[t * P : (t + 1) * P, :])
                bt = cpool.tile([P, O], dt.bfloat16, tag="bt", name=f"bt_{t}", bufs=6)
                nc.gpsimd.indirect_dma_start(
                    out=bt[:], out_offset=None,
                    in_=recv_all[:],
                    in_offset=bass.IndirectOffsetOnAxis(ap=bidx_sb[:, t : t + 1], axis=0),
                    bounds_check=EPC * N_CORES * C4 - 1,
                    oob_is_err=False,
                )
                nc.vector.tensor_add(at[:], at[:], bt[:])
                nc.scalar.dma_start(out[t * P : (t + 1) * P, :], at[:])

            if debug:
                nc.sync.dma_start(dbg_recv[:], recv_all[:])
                nc.sync.dma_start(dbg_cw[:], cw_sb[:])

    return out


def kernel(x, Wg, W1, b1, W2, b2):
    global LAST_EXEC_NS, LAST_RESULTS, LAST_PLAN
    x = np.ascontiguousarray(np.asarray(x, np.float32))
    Wg = np.ascontiguousarray(np.asarray(Wg, np.float32))
    W1 = np.ascontiguousarray(np.asarray(W1, np.float32))
    b1 = np.ascontiguousarray(np.asarray(b1, np.float32))
    W2 = np.ascontiguousarray(np.asarray(W2, np.float32))
    b2 = np.ascontiguousarray(np.asarray(b2, np.float32))

    B, D = x.shape
    E, _, H = W1.shape
    O = W2.shape[2]
    EPC = E // N_CORES

    pl = _plan(x, Wg)
    CAP, C4 = pl["CAP"], pl["C4"]
    S = EPC * CAP

    add_b1 = bool(np.any(b1))
    add_b2 = bool(np.any(b2))
    if add_b2:
        assert np.all(b2 == b2[0]), "per-expert nonzero b2 not supported"

    nc = bacc.Bacc("TRN2", target_bir_lowering=False, debug=False, num_devices=N_CORES)
    _build(nc, D, H, O, E, EPC, CAP, C4, pl["T_B"], add_b1, add_b2, debug=DEBUG)
    nc.compile()

    # ---- per-core input staging (pure data movement) ----
    xT_full = np.ascontiguousarray(x.T)  # [D, B]
    in_maps = []
    for c in range(N_CORES):
        toks = pl["slot_tok"][c]
        xTp = np.zeros((D, S), np.float32)
        real = toks >= 0
        xTp[:, real] = xT_full[:, toks[real]]
        xTp = np.ascontiguousarray(xTp.reshape(D // P, P, S).transpose(1, 0, 2))

        Wg_blocks = []
        for b in range(EPC):
            e = EPC * c + b
            perm = np.concatenate([[e], [j for j in range(E) if j != e]])
            Wg_blocks.append(Wg[:, perm].reshape(D // P, P, E).transpose(1, 0, 2))
        Wg_c = np.ascontiguousarray(np.stack(Wg_blocks, axis=1), np.float32)

        W1_c = np.stack(
            [W1[EPC * c + b].reshape(D // P, P, H).transpose(1, 0, 2) for b in range(EPC)]
        )
        W2_c = np.stack(
            [W2[EPC * c + b].reshape(H // P, P, O).transpose(1, 0, 2) for b in range(EPC)]
        )
        b1_c = np.stack([b1[EPC * c + b].reshape(H // P, P).T for b in range(EPC)]).transpose(1, 0, 2)
        b2_c = np.broadcast_to(b2[0], (P, O)).copy() if add_b2 else np.zeros((P, O), np.float32)
        in_maps.append(
            {
                "xT": np.ascontiguousarray(xTp, np.float32),
                "Wg": Wg_c,
                "W1": np.ascontiguousarray(W1_c, np.float32),
                "W2": np.ascontiguousarray(W2_c, np.float32),
                "b1": np.ascontiguousarray(b1_c, np.float32),
                "b2": np.ascontiguousarray(b2_c, np.float32),
                "sidx": np.ascontiguousarray(
                    pl["s_scat"][c].reshape(EPC, -1, P).transpose(2, 0, 1), np.int32
                ),
                "bidx": np.ascontiguousarray(
                    pl["b_idx"][c].reshape(-1, P).T, np.int32
                ),
            }
        )

    kwargs = {}
    if TRACE:
        import types

        try:
            import antenv  # noqa: F401
            from trn_agent_boot.trn_boot import _ntff_profile_via_ctypes

            hook = _ntff_profile_via_ctypes("/opt/axon/libaxon_pjrt.so")
            mod = types.ModuleType("antenv.axon_hooks")
            mod.get_axon_ntff_profile_hook = lambda: hook
            mod.set_axon_ntff_profile_hook = lambda h: None
            sys.modules.setdefault("antenv.axon_hooks", mod)
            kwargs["trace"] = True
        except Exception as e:  # pragma: no cover
            print("trace hook unavailable:", e)

    res = run_bass_kernel_spmd(nc, in_maps, core_ids=list(range(N_CORES)), **kwargs)
    LAST_EXEC_NS = res.exec_time_ns
    LAST_RESULTS = res.results
    LAST_PLAN = pl

    final = np.zeros((B, O), np.float32)
    for c in range(N_CORES):
        o = res.results[c]["out"]
        rows = np.array([sr for sr, _ in pl["A_rows"][c]], np.int64)
        tokens = np.array([t for _, t in pl["A_rows"][c]], np.int64)
        final[tokens] = o[rows]
    return final
